# revision 12
# baseline (speedup 1.0000x reference)
"""Trainium2 Bass kernel for nn_ProjectionLayer: mean-pool + projection +
L2-normalize + cosine-sim matrix / pairwise-distance denominator.

Reference math (fp32):
    g = mean(features, axis=2) @ W.T + bias        # [b, out_c]
    g = g / max(||g||_row, 1e-12)                  # L2 normalize rows
    sim = g @ g.T                                  # [b, b]
    dist = ||g + 1e-6||_row                        # [b]
    out = sim / max(dist_i, dist_j, 1e-8)

v2 strategy (HBM-bound problem; ~315 GB/s/core achievable):
  * fp16 on device: host casts features/W to fp16 (tolerance 2e-2; fp16
    keeps the result at ~1e-3). Halves the DMA stream phase.
  * The 1/196 mean scaling is absorbed by the L2 normalization (scale
    invariant), with bias scaled by 196 on device.
  * Pooling: two fp16 tensor_tensor folds (DVE 2x mode) + one reduce_sum
    of 49, instead of a 1x-mode reduce of 196 (which would be the
    bottleneck at fp16 stream rates).
  * Projection in [o, b] orientation (lhsT = W^T chunks loaded directly
    from a host-pretransposed W; no on-device weight transposes), in two
    row-halves so the first AllGather hides under the feature stream.
  * dist handled via first-order expansion 1/dist = 1 - 1e-6*sum(gn)
    (exact to ~6e-12), recomputed locally from the gathered fp16 gn --
    no sqrt/reciprocal chain and nothing extra gathered.
"""

import sys

if "/opt/trn_rl_repo" not in sys.path:
    sys.path.insert(0, "/opt/trn_rl_repo")

import numpy as np

# Problem shapes (hardcoded per contract)
B_FULL = 512     # batch
C_IN = 2048      # in channels
T_POOL = 196     # pooled (time) dim
O_OUT = 512      # out channels
N_CORES = 8

PD_EPS = 1e-6
DENO_EPS = 1e-8


def build_kernel(b_full, c_in, t_pool, o_out, n_cores, bg=8, cpp=8):
    """Emit the Bass module (SPMD program, identical on every core).

    cpp = channels per partition in the feature layout: partition p of
    chunk k holds channels c = span*k + cpp*p + j (j in [0, cpp)), so each
    DMA descriptor is a cpp*t_pool*2-byte contiguous run. The interleave is
    matched on the weight side by loading W^T rows in the same order.
    """
    import concourse.mybir as mybir
    import concourse.tile as tile
    from concourse import bacc
    from concourse.masks import make_identity

    f32 = mybir.dt.float32
    f16 = mybir.dt.float16
    AL = mybir.AluOpType
    AF = mybir.ActivationFunctionType

    bc = b_full // n_cores          # batch rows per core (64)
    span = 128 * cpp                # channels per chunk (1024)
    nk = c_in // span               # channel chunks (2)
    oc = o_out // 128               # out-channel chunks (4)
    nh = 2                          # row halves for AG pipelining
    hb = bc // nh                   # rows per half (32)
    th = t_pool // 2                # 98
    tq = 48                         # even fold-2 width (2x49 = 98 = 2*48 + 2)
    # batch groups (start, size): taper the end so the final DVE fold
    # chains (which serialize after the last feature DMA) are short
    sizes = [8, 8, 8, 8, 8, 8, 8, 4, 2, 2]
    assert sum(sizes) == bc and c_in % span == 0 and t_pool == 196
    groups = []
    off = 0
    for s in sizes:
        groups.append((off, s))
        off += s
    half_last = {hb: 0, bc: 1}      # group end offset -> half index

    nc = bacc.Bacc("TRN2", target_bir_lowering=False, debug=False,
                   enable_asserts=False, num_devices=n_cores)
    feat = nc.dram_tensor("features", [bc, c_in, t_pool], f16,
                          kind="ExternalInput").ap()
    wt_in = nc.dram_tensor("wt", [c_in, o_out], f16, kind="ExternalInput").ap()
    bias_in = nc.dram_tensor("bias", [o_out, 1], f32, kind="ExternalInput").ap()
    out_d = nc.dram_tensor("out", [bc, b_full], f32, kind="ExternalOutput").ap()

    with tile.TileContext(nc) as tc:
        with (
            tc.tile_pool(name="const", bufs=1) as constp,
            tc.tile_pool(name="wtp", bufs=1) as wtp,
            tc.tile_pool(name="featp", bufs=3) as fp,
            tc.tile_pool(name="foldp", bufs=2) as fo,
            tc.tile_pool(name="lhsp", bufs=1) as lp,
            tc.tile_pool(name="postp", bufs=1) as pp,
            tc.tile_pool(name="psA", bufs=1, space="PSUM") as psA,
            tc.tile_pool(name="psB", bufs=1, space="PSUM") as psB,
            tc.tile_pool(name="dram", bufs=1, space="DRAM") as dp,
        ):
            # ---- constants ----
            ident = constp.tile([128, 128], f32, name="ident")
            make_identity(nc, ident)
            ones_p = constp.tile([128, 1], f32, name="ones_p")     # col-sum lhsT
            nc.vector.memset(ones_p, 1.0)
            ones_p16 = constp.tile([128, 1], f16, name="ones_p16")
            nc.vector.memset(ones_p16, 1.0)
            ones_r = constp.tile([1, 128], f32, name="ones_r")     # bcast lhsT
            nc.vector.memset(ones_r, 1.0)
            ones_r16 = constp.tile([1, 128], f16, name="ones_r16")
            nc.vector.memset(ones_r16, 1.0)
            zeros128 = constp.tile([128, 128], f16, name="zeros128")
            nc.vector.memset(zeros128, 0.0)

            # bias as [128, oc] (partition p, chunk m) <- bias[o=m*128+p], x196
            bias_sb = constp.tile([128, oc], f32, name="bias_sb")
            nc.sync.dma_start(
                bias_sb[:],
                bias_in.rearrange("(m p) one -> p (m one)", p=128))
            bias196 = constp.tile([128, oc], f32, name="bias196")
            nc.scalar.mul(bias196[:], bias_sb[:], float(t_pool))

            # W^T chunks, interleave-matched: wtl[k][p, j, o] = WT[span*k+cpp*p+j, o]
            wtl = []
            for k in range(nk):
                w = wtp.tile([128, cpp, o_out], f16, name=f"wtl{k}")
                nc.sync.dma_start(
                    w[:],
                    wt_in[k * span:(k + 1) * span, :].rearrange(
                        "(p j) o -> p j o", j=cpp))
                wtl.append(w)

            # ---- AllGather buffers (one pair per half) ----
            agi = [dp.tile([b_full, hb], f16, name=f"agi{h}") for h in range(nh)]
            ago = [dp.tile([b_full * n_cores, hb], f16, name=f"ago{h}",
                           addr_space="Shared") for h in range(nh)]

            # persistent tiles
            p4 = [lp.tile([128, bc, cpp], f32, name=f"p4_{k}") for k in range(nk)]
            p4h = [[lp.tile([128, hb, cpp], f16, name=f"p4h{k}_{h}")
                    for h in range(nh)] for k in range(nk)]
            gnT = pp.tile([128, oc, nh, hb], f16, name="gnT")  # local normalized^T
            rdl_row = pp.tile([1, bc], f32, name="rdl_row")    # local 1/dist row
            gr = [pp.tile([128, n_cores, oc, hb], f16, name=f"gr{h}")
                  for h in range(nh)]

            # PSUM tiles
            gpsh = [psA.tile([128, oc * hb], f32, name=f"gps{h}")
                    for h in range(nh)]               # [o-part, (m, b)] per half
            sps = psB.tile([bc, n_cores, nh, hb], f32, name="sps")
            n2s1 = psA.tile([1, 2 * hb], f32, name="n2s1", tag="n2s1")
            replps = psA.tile([128, hb], f32, name="replps", tag="repl")
            s1all = psA.tile([1, n_cores, nh, hb], f32, name="s1all", tag="s1a")
            rdlT = psA.tile([bc, 1], f32, name="rdlT", tag="rdlT")
            rdjrep = psB.tile([bc, n_cores, nh, hb], f32, name="rdjrep", tag="rdj")
            n2ps = n2s1[:, 0:hb]
            s1ps = n2s1[:, hb:2 * hb]

            def gcol(m, h):      # gps column slice for (o-chunk m, half h)
                return gpsh[h][:, m * hb:(m + 1) * hb]

            # ---- streaming phase: DMA + fold pooling, per (batch group, k) ----
            idma = 0
            for gstart, gsz in groups:
                for k in range(nk):
                    ft = fp.tile([128, gsz, cpp, t_pool], f16, name="ft")
                    src = feat[gstart:gstart + gsz,
                               k * span:(k + 1) * span, :].rearrange(
                                   "b (p j) t -> p b j t", j=cpp)
                    dma_eng = nc.sync if idma % 2 == 0 else nc.scalar
                    dma_eng.dma_start(ft[:], src)
                    idma += 1
                    h1 = fo.tile([128, gsz, cpp, th], f16, name="h1")
                    nc.vector.tensor_add(h1[:], ft[:, :, :, 0:th],
                                         ft[:, :, :, th:t_pool])
                    h2 = fo.tile([128, gsz, cpp, tq + 1], f16, name="h2")
                    nc.vector.tensor_add(h2[:, :, :, 0:tq], h1[:, :, :, 0:tq],
                                         h1[:, :, :, tq:2 * tq])
                    nc.vector.tensor_add(h2[:, :, :, tq:tq + 1],
                                         h1[:, :, :, 2 * tq:2 * tq + 1],
                                         h1[:, :, :, 2 * tq + 1:2 * tq + 2])
                    nc.vector.reduce_sum(p4[k][:, gstart:gstart + gsz, :],
                                         h2[:], axis=mybir.AxisListType.X)
                    # cast this k-chunk for the half as soon as its rows done
                    if gstart + gsz in half_last:
                        h = half_last[gstart + gsz]
                        nc.vector.tensor_copy(p4h[k][h][:],
                                              p4[k][:, h * hb:h * hb + hb, :])

                # ---- per-half: project, normalize, AllGather ----
                if gstart + gsz in half_last:
                    h = half_last[gstart + gsz]
                    r0 = h * hb
                    # Pre-open the PSUM bank with one start=True all-zeros
                    # matmul covering the whole tile (start clears the whole
                    # bank's has_written bits, so per-region start flags must
                    # not interleave; this way every real matmul is a pure
                    # accumulate in any order).
                    nc.tensor.matmul(gpsh[h][:, 0:oc * hb], zeros128[:],
                                     zeros128[:], start=True, stop=False,
                                     skip_group_check=True)
                    # projection: gps[o, b] += WT[c, o].T @ pool[c, b]
                    # (k outer: the k=0 matmuls only need the k=0 cast)
                    for k in range(nk):
                        for j in range(cpp):
                            for m in range(oc):
                                nc.tensor.matmul(
                                    gcol(m, h),
                                    wtl[k][:, j, m * 128:(m + 1) * 128],
                                    p4h[k][h][:, :, j],
                                    start=False,
                                    stop=(k == nk - 1 and j == cpp - 1),
                                    skip_group_check=True)
                    # normalize: gb = gps + 196*bias; n2/s1 via ones-matmuls
                    gb = []
                    for m in range(oc):
                        gbm = pp.tile([128, hb], f32, name=f"gb{m}")
                        nc.vector.tensor_scalar_add(gbm[:], gcol(m, h),
                                                    bias196[:, m:m + 1])
                        gb.append(gbm)
                    sq = []
                    for m in range(oc):
                        sqm = pp.tile([128, hb], f32, name=f"sq{m}")
                        nc.scalar.square(sqm[:], gb[m][:])
                        sq.append(sqm)
                    for m in range(oc):
                        nc.tensor.matmul(n2ps, ones_p[:], sq[m][:],
                                         start=(m == 0), stop=(m == oc - 1))
                    for m in range(oc):
                        nc.tensor.matmul(s1ps, ones_p[:], gb[m][:],
                                         start=(m == 0), stop=(m == oc - 1))
                    nrm = pp.tile([1, hb], f32, name="nrm", tag="nrm")
                    nc.scalar.sqrt(nrm[:], n2ps)
                    rinv = pp.tile([1, hb], f32, name="rinv", tag="rinv")
                    nc.vector.reciprocal(rinv[:], nrm[:])
                    # local 1/dist = 1 - 1e-6 * (s1 * rinv)  (first-order exact)
                    s1n = pp.tile([1, hb], f32, name="s1n", tag="s1n")
                    nc.vector.tensor_mul(s1n[:], s1ps, rinv[:])
                    nc.vector.tensor_scalar(rdl_row[:, r0:r0 + hb], s1n[:],
                                            -PD_EPS, 1.0,
                                            op0=AL.mult, op1=AL.add)
                    # gnT = gb * rinv (broadcast rinv over partitions via PE)
                    nc.tensor.matmul(replps[:], ones_r[:], rinv[:],
                                     start=True, stop=True)
                    for m in range(oc):
                        nc.vector.tensor_mul(gnT[:, m, h, :], gb[m][:],
                                             replps[:])
                    # ship normalized half: agi[h][m*128+p, b] = gnT[p, m, h, b]
                    nc.sync.dma_start(
                        agi[h].rearrange("(m p) b -> p m b", p=128),
                        gnT[:, :, h, :])
                    nc.gpsimd.collective_compute(
                        "AllGather", AL.bypass,
                        replica_groups=[list(range(n_cores))],
                        ins=[agi[h].opt()], outs=[ago[h].opt()],
                    )

            # ---- gathered side: load, column sums, sim, denominator ----
            for h in range(nh):
                nc.sync.dma_start(
                    gr[h][:],
                    ago[h].rearrange("(r m p) b -> p r m b", p=128, m=oc))
            # s1all[j] = sum_o gn_all[o, j]  (global col order j = r*64+h*32+b)
            for h in range(nh):
                for m in range(oc):
                    nc.tensor.matmul(s1all[:, :, h, :], ones_p16[:],
                                     gr[h][:, :, m, :],
                                     start=(m == 0), stop=(m == oc - 1))
            rdist = pp.tile([1, n_cores, nh, hb], f32, name="rdist")
            # local rdl as column [bc, 1] via PE transpose
            nc.tensor.transpose(rdlT[:], rdl_row[:], ident[:1, :1])
            rdl_col = pp.tile([bc, 1], f32, name="rdl_col")
            nc.vector.tensor_copy(rdl_col[:], rdlT[:])

            # per-half (h=0 work overlaps the h=1 AllGather):
            # sim, 1/deno = min(rdl_i, rdist_j, 1/eps), out = sim * (1/deno)
            outsb = pp.tile([bc, n_cores, nh, hb], f32, name="outsb")
            rden = pp.tile([bc, n_cores, nh, hb], f32, name="rden")
            for h in range(nh):
                nc.vector.tensor_scalar(rdist[:, :, h, :], s1all[:, :, h, :],
                                        -PD_EPS, 1.0, op0=AL.mult, op1=AL.add)
                for m in range(oc):
                    nc.tensor.matmul(sps[:, :, h, :],
                                     gnT[:, m, :, :], gr[h][:, :, m, :],
                                     start=(m == 0), stop=(m == oc - 1))
                nc.tensor.matmul(rdjrep[:, :, h, :], ones_r[:, :bc],
                                 rdist[:, :, h, :], start=True, stop=True)
                nc.vector.tensor_scalar(rden[:, :, h, :], rdjrep[:, :, h, :],
                                        rdl_col[:], 1.0 / DENO_EPS,
                                        op0=AL.min, op1=AL.min)
                nc.vector.tensor_mul(outsb[:, :, h, :], sps[:, :, h, :],
                                     rden[:, :, h, :])
            nc.sync.dma_start(out_d[:], outsb[:])

    nc.compile()
    return nc


_NC_CACHE = {}


def _get_nc():
    key = (B_FULL, C_IN, T_POOL, O_OUT, N_CORES)
    if key not in _NC_CACHE:
        _NC_CACHE[key] = build_kernel(*key)
    return _NC_CACHE[key]


def _run(features, W, bias, trace=False):
    from concourse.bass_utils import run_bass_kernel_spmd

    feats16 = np.ascontiguousarray(np.asarray(features)).astype(np.float16)
    wt16 = np.ascontiguousarray(
        np.asarray(W, dtype=np.float32).T.astype(np.float16))
    bias_np = np.ascontiguousarray(
        np.asarray(bias, dtype=np.float32).reshape(O_OUT, 1))
    bc = B_FULL // N_CORES

    nc = _get_nc()
    in_maps = [
        {"features": feats16[r * bc:(r + 1) * bc], "wt": wt16, "bias": bias_np}
        for r in range(N_CORES)
    ]
    res = run_bass_kernel_spmd(nc, in_maps, core_ids=list(range(N_CORES)),
                               trace=trace)
    out = np.concatenate([res.results[r]["out"] for r in range(N_CORES)], axis=0)
    return out, res.exec_time_ns


def kernel(features, W, bias):
    out, _ = _run(features, W, bias)
    return out


# revision 13
# speedup vs baseline: 1.0210x; 1.0210x over previous
"""Trainium2 Bass kernel for nn_ProjectionLayer: mean-pool + projection +
L2-normalize + cosine-sim matrix / pairwise-distance denominator.

Reference math (fp32):
    g = mean(features, axis=2) @ W.T + bias        # [b, out_c]
    g = g / max(||g||_row, 1e-12)                  # L2 normalize rows
    sim = g @ g.T                                  # [b, b]
    dist = ||g + 1e-6||_row                        # [b]
    out = sim / max(dist_i, dist_j, 1e-8)

Strategy (HBM-bound problem; ~315 GB/s/core sustained):
  * fp16 on device: host casts features/W to fp16 (tolerance 2e-2; result
    lands at ~5.7e-4). Halves the DMA stream phase vs fp32.
  * The 1/196 mean scaling is absorbed by the L2 normalization (scale
    invariant); bias is scaled by 196 on device instead.
  * Pooling: two fp16 tensor_tensor folds (DVE 2x mode) + one reduce_sum
    of 49 (reduce_sum alone is 1x mode and would be the bottleneck).
  * Projection in [o, b] orientation (lhsT = W^T chunks DMA'd from a
    host-pretransposed W; no on-device weight transposes). The PSUM bank
    is pre-opened with one start=True all-zeros matmul because start=True
    clears has_written bits for the whole bank (so per-region start flags
    must not interleave).
  * Rows split 48/16: the 48-row chunk's project/normalize/AllGather all
    hide under the feature stream; only the 16-row chunk's chain + its
    small AllGather are exposed at the tail. Batch groups taper at the
    end so the final DVE fold chains are short.
  * dist via first-order expansion 1/dist = 1 - 1e-6*sum(gn) (exact to
    ~6e-12), recomputed locally from the gathered fp16 gn -- no
    sqrt/reciprocal chain on [1,512] and nothing extra gathered.
"""

import sys

if "/opt/trn_rl_repo" not in sys.path:
    sys.path.insert(0, "/opt/trn_rl_repo")

import numpy as np

# Problem shapes (hardcoded per contract)
B_FULL = 512     # batch
C_IN = 2048      # in channels
T_POOL = 196     # pooled (time) dim
O_OUT = 512      # out channels
N_CORES = 8

PD_EPS = 1e-6
DENO_EPS = 1e-8


def build_kernel(b_full, c_in, t_pool, o_out, n_cores, cpp=8):
    """Emit the Bass module (SPMD program, identical on every core).

    cpp = channels per partition in the feature layout: partition p of
    chunk k holds channels c = span*k + cpp*p + j (j in [0, cpp)), so each
    DMA descriptor is a cpp*t_pool*2-byte contiguous run. The interleave is
    matched on the weight side by loading W^T rows in the same order.
    """
    import concourse.mybir as mybir
    import concourse.tile as tile
    from concourse import bacc
    from concourse.masks import make_identity

    f32 = mybir.dt.float32
    f16 = mybir.dt.float16
    AL = mybir.AluOpType

    bc = b_full // n_cores          # batch rows per core (64)
    span = 128 * cpp                # channels per chunk (1024)
    nk = c_in // span               # channel chunks (2)
    oc = o_out // 128               # out-channel chunks (4)
    nh = 2                          # row chunks for AG pipelining
    hbs = [48, 16]                  # rows per chunk (uneven: small one last)
    hoff = [0, 48]
    th = t_pool // 2                # 98
    tq = 48                         # even fold-2 width (98 = 2*48 + 2)
    # batch groups (start, size): taper the end so the final DVE fold
    # chains (which serialize after the last feature DMA) are short
    sizes = [8, 8, 8, 8, 8, 8, 8, 4, 2, 2]
    assert sum(sizes) == bc and c_in % span == 0 and t_pool == 196
    groups = []
    off = 0
    for s in sizes:
        groups.append((off, s))
        off += s
    half_last = {hoff[0] + hbs[0]: 0, bc: 1}   # group end offset -> chunk
    assert all(any(g + s == e for g, s in groups) for e in half_last)

    nc = bacc.Bacc("TRN2", target_bir_lowering=False, debug=False,
                   enable_asserts=False, num_devices=n_cores)
    feat = nc.dram_tensor("features", [bc, c_in, t_pool], f16,
                          kind="ExternalInput").ap()
    wt_in = nc.dram_tensor("wt", [c_in, o_out], f16, kind="ExternalInput").ap()
    bias_in = nc.dram_tensor("bias", [o_out, 1], f32, kind="ExternalInput").ap()
    out_d = nc.dram_tensor("out", [bc, b_full], f32, kind="ExternalOutput").ap()

    with tile.TileContext(nc) as tc:
        with (
            tc.tile_pool(name="const", bufs=1) as constp,
            tc.tile_pool(name="wtp", bufs=1) as wtp,
            tc.tile_pool(name="featp", bufs=3) as fp,
            tc.tile_pool(name="foldp", bufs=2) as fo,
            tc.tile_pool(name="lhsp", bufs=1) as lp,
            tc.tile_pool(name="postp", bufs=1) as pp,
            tc.tile_pool(name="psA", bufs=1, space="PSUM") as psA,
            tc.tile_pool(name="psB", bufs=1, space="PSUM") as psB,
            tc.tile_pool(name="dram", bufs=1, space="DRAM") as dp,
        ):
            # ---- constants ----
            ident = constp.tile([128, 128], f32, name="ident")
            make_identity(nc, ident)
            ones_p = constp.tile([128, 1], f32, name="ones_p")     # col-sum lhsT
            nc.vector.memset(ones_p, 1.0)
            ones_p16 = constp.tile([128, 1], f16, name="ones_p16")
            nc.vector.memset(ones_p16, 1.0)
            ones_r = constp.tile([1, 128], f32, name="ones_r")     # bcast lhsT
            nc.vector.memset(ones_r, 1.0)
            zeros128 = constp.tile([128, 128], f16, name="zeros128")
            nc.vector.memset(zeros128, 0.0)

            # bias as [128, oc] (partition p, chunk m) <- bias[o=m*128+p], x196
            bias_sb = constp.tile([128, oc], f32, name="bias_sb")
            nc.sync.dma_start(
                bias_sb[:],
                bias_in.rearrange("(m p) one -> p (m one)", p=128))
            bias196 = constp.tile([128, oc], f32, name="bias196")
            nc.scalar.mul(bias196[:], bias_sb[:], float(t_pool))

            # W^T chunks, interleave-matched: wtl[k][p, j, o] = WT[span*k+cpp*p+j, o]
            wtl = []
            for k in range(nk):
                w = wtp.tile([128, cpp, o_out], f16, name=f"wtl{k}")
                nc.sync.dma_start(
                    w[:],
                    wt_in[k * span:(k + 1) * span, :].rearrange(
                        "(p j) o -> p j o", j=cpp))
                wtl.append(w)

            # ---- AllGather buffers (one pair per row chunk) ----
            agi = [dp.tile([o_out, hbs[h]], f16, name=f"agi{h}")
                   for h in range(nh)]
            ago = [dp.tile([o_out * n_cores, hbs[h]], f16, name=f"ago{h}",
                           addr_space="Shared") for h in range(nh)]

            # persistent tiles
            p4 = [lp.tile([128, bc, cpp], f32, name=f"p4_{k}") for k in range(nk)]
            p4h = [[lp.tile([128, hbs[h], cpp], f16, name=f"p4h{k}_{h}")
                    for h in range(nh)] for k in range(nk)]
            gnT = pp.tile([128, oc, bc], f16, name="gnT")   # local normalized^T
            rdl_row = pp.tile([1, bc], f32, name="rdl_row")  # local 1/dist row
            gr = [pp.tile([128, n_cores, oc, hbs[h]], f16, name=f"gr{h}")
                  for h in range(nh)]

            # PSUM tiles
            gpsh = [psA.tile([128, oc * hbs[h]], f32, name=f"gps{h}")
                    for h in range(nh)]              # [o-part, (m, b)]
            sps = psB.tile([bc, n_cores, bc], f32, name="sps")
            n2s1 = psA.tile([1, 2 * hbs[0]], f32, name="n2s1", tag="n2s1")
            replps = psA.tile([128, hbs[0]], f32, name="replps", tag="repl")
            s1all = psA.tile([1, n_cores, bc], f32, name="s1all", tag="s1a")
            rdlT = psA.tile([bc, 1], f32, name="rdlT", tag="rdlT")
            rdjrep = psB.tile([bc, n_cores, bc], f32, name="rdjrep", tag="rdj")

            def gcol(m, h):      # gps column slice for (o-chunk m, row chunk h)
                return gpsh[h][:, m * hbs[h]:(m + 1) * hbs[h]]

            # ---- streaming phase: DMA + fold pooling, per (batch group, k) ----
            idma = 0
            for gstart, gsz in groups:
                for k in range(nk):
                    ft = fp.tile([128, gsz, cpp, t_pool], f16, name="ft")
                    src = feat[gstart:gstart + gsz,
                               k * span:(k + 1) * span, :].rearrange(
                                   "b (p j) t -> p b j t", j=cpp)
                    dma_eng = nc.sync if idma % 2 == 0 else nc.scalar
                    dma_eng.dma_start(ft[:], src)
                    idma += 1
                    h1 = fo.tile([128, gsz, cpp, th], f16, name="h1")
                    nc.vector.tensor_add(h1[:], ft[:, :, :, 0:th],
                                         ft[:, :, :, th:t_pool])
                    h2 = fo.tile([128, gsz, cpp, tq + 1], f16, name="h2")
                    nc.vector.tensor_add(h2[:, :, :, 0:tq], h1[:, :, :, 0:tq],
                                         h1[:, :, :, tq:2 * tq])
                    nc.vector.tensor_add(h2[:, :, :, tq:tq + 1],
                                         h1[:, :, :, 2 * tq:2 * tq + 1],
                                         h1[:, :, :, 2 * tq + 1:2 * tq + 2])
                    nc.vector.reduce_sum(p4[k][:, gstart:gstart + gsz, :],
                                         h2[:], axis=mybir.AxisListType.X)
                    # cast this k-chunk as soon as the row chunk's rows done
                    if gstart + gsz in half_last:
                        h = half_last[gstart + gsz]
                        nc.vector.tensor_copy(
                            p4h[k][h][:],
                            p4[k][:, hoff[h]:hoff[h] + hbs[h], :])

                # ---- per row chunk: project, normalize, AllGather ----
                if gstart + gsz in half_last:
                    h = half_last[gstart + gsz]
                    hb = hbs[h]
                    r0 = hoff[h]
                    # Pre-open the PSUM bank: start=True clears has_written
                    # for the whole bank, so do it once with zeros and let
                    # every real matmul be a pure accumulate in any order.
                    nc.tensor.matmul(gpsh[h][:, 0:oc * hb], zeros128[:],
                                     wtl[0][:, 0, 0:oc * hb], start=True,
                                     stop=False, skip_group_check=True)
                    # projection: gps[o, b] += WT[c, o].T @ pool[c, b]
                    # (k outer: the k=0 matmuls only need the k=0 cast)
                    for k in range(nk):
                        for j in range(cpp):
                            for m in range(oc):
                                nc.tensor.matmul(
                                    gcol(m, h),
                                    wtl[k][:, j, m * 128:(m + 1) * 128],
                                    p4h[k][h][:, :, j],
                                    start=False,
                                    stop=(k == nk - 1 and j == cpp - 1),
                                    skip_group_check=True)
                    # normalize: gb = gps + 196*bias; n2/s1 via ones-matmuls
                    n2ps = n2s1[:, 0:hb]
                    s1ps = n2s1[:, hbs[0]:hbs[0] + hb]
                    gb = []
                    for m in range(oc):
                        gbm = pp.tile([128, hbs[0]], f32, name=f"gb{m}")
                        nc.vector.tensor_scalar_add(gbm[:, 0:hb], gcol(m, h),
                                                    bias196[:, m:m + 1])
                        gb.append(gbm)
                    sq = []
                    for m in range(oc):
                        sqm = pp.tile([128, hbs[0]], f32, name=f"sq{m}")
                        nc.scalar.square(sqm[:, 0:hb], gb[m][:, 0:hb])
                        sq.append(sqm)
                    for m in range(oc):
                        nc.tensor.matmul(n2ps, ones_p[:], sq[m][:, 0:hb],
                                         start=(m == 0), stop=(m == oc - 1))
                    for m in range(oc):
                        nc.tensor.matmul(s1ps, ones_p[:], gb[m][:, 0:hb],
                                         start=(m == 0), stop=(m == oc - 1))
                    nrm = pp.tile([1, hbs[0]], f32, name="nrm")
                    nc.scalar.sqrt(nrm[:, 0:hb], n2ps)
                    rinv = pp.tile([1, hbs[0]], f32, name="rinv")
                    nc.vector.reciprocal(rinv[:, 0:hb], nrm[:, 0:hb])
                    # local 1/dist = 1 - 1e-6 * (s1 * rinv)  (first-order)
                    s1n = pp.tile([1, hbs[0]], f32, name="s1n")
                    nc.vector.tensor_mul(s1n[:, 0:hb], s1ps, rinv[:, 0:hb])
                    nc.vector.tensor_scalar(rdl_row[:, r0:r0 + hb],
                                            s1n[:, 0:hb], -PD_EPS, 1.0,
                                            op0=AL.mult, op1=AL.add)
                    # gnT = gb * rinv (broadcast rinv over partitions via PE)
                    nc.tensor.matmul(replps[:, 0:hb], ones_r[:], rinv[:, 0:hb],
                                     start=True, stop=True)
                    for m in range(oc):
                        nc.vector.tensor_mul(gnT[:, m, r0:r0 + hb],
                                             gb[m][:, 0:hb], replps[:, 0:hb])
                    # ship the normalized chunk: agi[h][m*128+p, b]
                    nc.sync.dma_start(
                        agi[h].rearrange("(m p) b -> p m b", p=128),
                        gnT[:, :, r0:r0 + hb])
                    nc.gpsimd.collective_compute(
                        "AllGather", AL.bypass,
                        replica_groups=[list(range(n_cores))],
                        ins=[agi[h].opt()], outs=[ago[h].opt()],
                    )

            # ---- gathered side: load, column sums, sim, denominator ----
            for h in range(nh):
                nc.sync.dma_start(
                    gr[h][:],
                    ago[h].rearrange("(r m p) b -> p r m b", p=128, m=oc))
            # local rdl as column [bc, 1] via PE transpose
            nc.tensor.transpose(rdlT[:], rdl_row[:], ident[:1, :1])
            rdl_col = pp.tile([bc, 1], f32, name="rdl_col")
            nc.vector.tensor_copy(rdl_col[:], rdlT[:])

            # per chunk h (h=0 work overlaps the h=1 AllGather):
            # s1all[j] = sum_o gn_all[o, j]; 1/dist_j = 1 - 1e-6*s1all[j]
            # sim[i, j] = sum_o gnT[o, i]*gn_all[o, j]
            # 1/deno = min(rdl_i, rdist_j, 1/eps); out = sim * (1/deno)
            rdist = pp.tile([1, n_cores, bc], f32, name="rdist")
            outsb = pp.tile([bc, n_cores, bc], f32, name="outsb")
            rden = pp.tile([bc, n_cores, bc], f32, name="rden")

            def csl(t, h):       # column slice (all ranks, chunk h) of [*, r, b]
                return t[:, :, hoff[h]:hoff[h] + hbs[h]]

            for h in range(nh):
                for m in range(oc):
                    nc.tensor.matmul(csl(s1all, h), ones_p16[:],
                                     gr[h][:, :, m, :],
                                     start=(m == 0), stop=(m == oc - 1))
                nc.vector.tensor_scalar(csl(rdist, h), csl(s1all, h),
                                        -PD_EPS, 1.0, op0=AL.mult, op1=AL.add)
                for m in range(oc):
                    nc.tensor.matmul(csl(sps, h),
                                     gnT[:, m, :], gr[h][:, :, m, :],
                                     start=(m == 0), stop=(m == oc - 1))
                nc.tensor.matmul(csl(rdjrep, h), ones_r[:, :bc],
                                 csl(rdist, h), start=True, stop=True)
                nc.vector.tensor_scalar(csl(rden, h), csl(rdjrep, h),
                                        rdl_col[:], 1.0 / DENO_EPS,
                                        op0=AL.min, op1=AL.min)
                nc.vector.tensor_mul(csl(outsb, h), csl(sps, h), csl(rden, h))
            nc.sync.dma_start(out_d[:], outsb[:])

    nc.compile()
    return nc


_NC_CACHE = {}


def _get_nc():
    key = (B_FULL, C_IN, T_POOL, O_OUT, N_CORES)
    if key not in _NC_CACHE:
        _NC_CACHE[key] = build_kernel(*key)
    return _NC_CACHE[key]


def _run(features, W, bias, trace=False):
    from concourse.bass_utils import run_bass_kernel_spmd

    feats16 = np.ascontiguousarray(np.asarray(features)).astype(np.float16)
    wt16 = np.ascontiguousarray(
        np.asarray(W, dtype=np.float32).T.astype(np.float16))
    bias_np = np.ascontiguousarray(
        np.asarray(bias, dtype=np.float32).reshape(O_OUT, 1))
    bc = B_FULL // N_CORES

    nc = _get_nc()
    in_maps = [
        {"features": feats16[r * bc:(r + 1) * bc], "wt": wt16, "bias": bias_np}
        for r in range(N_CORES)
    ]
    res = run_bass_kernel_spmd(nc, in_maps, core_ids=list(range(N_CORES)),
                               trace=trace)
    out = np.concatenate([res.results[r]["out"] for r in range(N_CORES)], axis=0)
    return out, res.exec_time_ns


def kernel(features, W, bias):
    out, _ = _run(features, W, bias)
    return out
